# revision 55
# baseline (speedup 1.0000x reference)
"""CondConv (MoE routed conv) Trainium2 Bass kernel — v4.

Sharding: core c -> samples [4*(c//2), 4*(c//2)+4), cout half ot = c%2.

Changes vs v3 (135992 ns):
  - All input DMA on the sync ring in strict priority order: gate const,
    x0 quarters, routing smalls, it0 slab pairs, x1, it1 slab pairs,
    x2, x3.  Slabs are loaded as expert PAIRS (host pre-interleaved) so
    the combine chain can chase arrivals; y output DMAs follow on the
    same ring.
  - No gpsimd DMA ring (its swdge-init memsets started the measured
    exec window ~1.1us early); nothing executes before the first
    DIRECT2D.
  - GAP for s0 runs as 4 quarter reduces on DVE with bf16 output
    (2x perf mode), chasing the x0 quarter DMAs; routing runs ~7us
    earlier than v3.
  - s0 combine chains chase the slab-pair arrivals; remaining samples
    use pure-DVE chains (ACT keeps epilogue slack).
  - Routing matmuls for s1..s3 are woven into the conv phase-A stream.
"""

import sys

sys.path.insert(0, "/opt/trn_rl_repo")

import ml_dtypes
import numpy as np

import concourse.bass as bass  # noqa: F401
import concourse.mybir as mybir
import concourse.tile as tile
from concourse import bacc
from concourse.bass_utils import run_bass_kernel_spmd

F32 = mybir.dt.float32
BF16 = mybir.dt.bfloat16
AF = mybir.ActivationFunctionType
ALU = mybir.AluOpType
NPBF16 = ml_dtypes.bfloat16

B, CIN, H, W = 16, 256, 56, 56
E, COUT, KS = 8, 256, 3
NCORES = 8
SPC = 4
IT = CIN // 128
OT = COUT // 128
KHKW = KS * KS
HB = 8
WP = W + 2
PIX = H * W
BN_EPS = 1e-5
SLAB = KHKW * 128  # 1152
NPAIR = E // 2
NPA = 7
JUNK_A = 17
JUNK_M = 2
JUNK_B = 2
HHALF = H // 2
HWP = H * WP

_PROGRAM_CACHE = {}


def _build_program():
    nc = bacc.Bacc("TRN2", target_bir_lowering=False, debug=False)

    jc_d = nc.dram_tensor("jc", [128, 448], BF16, kind="ExternalInput")
    # x is partition-major: each partition's IT*H*WP elements contiguous,
    # so DMA descriptors are one long run per partition (quarters = flat
    # column ranges), not 116-byte rows.
    x_d = nc.dram_tensor("x", [SPC, 128, IT * H * WP], BF16, kind="ExternalInput")
    wt_d = nc.dram_tensor("wt", [IT, NPAIR, 128, 2 * SLAB], BF16, kind="ExternalInput")
    rwt_d = nc.dram_tensor("rwt", [128, IT, E], BF16, kind="ExternalInput")
    # cst cols 0:128 = ones (bcast lhsT), cols 128:128+E = routing bias
    cst_d = nc.dram_tensor("cst", [1, 128 + E], F32, kind="ExternalInput")
    bns_d = nc.dram_tensor("bns", [128, 1], F32, kind="ExternalInput")
    bnb_d = nc.dram_tensor("bnb", [128, 1], F32, kind="ExternalInput")
    y_d = nc.dram_tensor("y", [SPC, 128, H, W], BF16, kind="ExternalOutput")

    with tile.TileContext(nc) as tc:
        with (
            tc.tile_pool(name="xp", bufs=1) as xp,
            tc.tile_pool(name="cwp", bufs=1) as cwp,
            tc.tile_pool(name="wtp", bufs=1) as wtp,
            tc.tile_pool(name="outp", bufs=6) as outp,
            tc.tile_pool(name="smal", bufs=1) as smal,
            tc.tile_pool(name="psc", bufs=NPA, space="PSUM") as psc,
            tc.tile_pool(name="pss", bufs=1, space="PSUM") as pss,
        ):
            # ---- priority-ordered input DMA, all on the sync ring ----
            jc = smal.tile([128, 448], BF16, tag="jc")
            nc.sync.dma_start(jc[:], jc_d[:])

            x_sb = {}
            x_flat = {}
            for s in range(SPC):
                x_sb[s] = xp.tile([128, IT, H, WP], BF16, tag=f"x{s}", name=f"x{s}")
                x_flat[s] = x_sb[s][:].rearrange("p a b c -> p (a b c)")

            QF = HHALF * WP  # flat elems per (it, H-half) quarter
            # x0 row-thirds per it (6 pieces) for a finer GAP chase
            THIRDS = [(0, 18), (18, 37), (37, H)]

            def load_x_rows(s, it, r0, r1):
                c0 = it * HWP + r0 * WP
                c1 = it * HWP + r1 * WP
                nc.sync.dma_start(x_flat[s][:, c0:c1], x_d[s, :, c0:c1])

            def load_x_quarter(s, it, q):
                load_x_rows(s, it, q * HHALF, (q + 1) * HHALF)

            for it in range(IT):
                for r0, r1 in THIRDS:
                    load_x_rows(0, it, r0, r1)

            rwt_sb = smal.tile([128, IT, E], BF16, tag="rwt")
            nc.sync.dma_start(rwt_sb[:], rwt_d[:])
            cst_sb = smal.tile([1, 128 + E], F32, tag="cst")
            nc.sync.dma_start(cst_sb[:], cst_d[:])
            ones_sb = cst_sb[:, 0:128]
            rb_sb = cst_sb[:, 128 : 128 + E]

            pair_tiles = {}
            DG0 = 512

            def load_slab_pairs(it, split=False):
                for p in range(NPAIR):
                    t = wtp.tile(
                        [128, 2 * SLAB], BF16, tag=f"wt{it}{p}", name=f"wt{it}{p}"
                    )
                    if split:
                        # head cols [0:DG0) of both experts first (the DVE
                        # chain region -> no arrival chase), tails after
                        # (the PE diag region chases them).
                        tv = t[:].rearrange("q (e c) -> q e c", e=2)
                        sv = wt_d[it, p].rearrange("q (e c) -> q e c", e=2)
                        nc.sync.dma_start(tv[:, :, 0:DG0], sv[:, :, 0:DG0])
                        nc.sync.dma_start(tv[:, :, DG0:], sv[:, :, DG0:])
                    else:
                        nc.sync.dma_start(t[:], wt_d[it, p])
                    pair_tiles[it, p] = t

            def slab(it, e):
                t = pair_tiles[it, e // 2]
                off = (e % 2) * SLAB
                return t[:, off : off + SLAB]

            load_slab_pairs(0, split=True)
            load_slab_pairs(1)
            bns_sb = smal.tile([128, 1], F32, tag="bns")
            nc.sync.dma_start(bns_sb[:], bns_d[:])
            bnb_sb = smal.tile([128, 1], F32, tag="bnb")
            nc.sync.dma_start(bnb_sb[:], bnb_d[:])
            # x1 split in quarters so its GAP reduces can chase
            for it in range(IT):
                for q in range(2):
                    load_x_quarter(1, it, q)
            nc.sync.dma_start(x_flat[2][:], x_d[2])
            nc.sync.dma_start(x_flat[3][:], x_d[3])

            # ---- ACT table preload (gated on jc, not t=0). Silu is the
            # ONLY table the kernel ever uses: routing computes
            # sigmoid(z) as silu(z)/z so the ACT engine never swaps
            # tables (a swap costs 1.28us and landed on the routing
            # critical path).
            sig_dummy = smal.tile([1, 8], BF16, tag="sigd")
            nc.scalar.activation(sig_dummy[:], jc[0:1, 0:8], AF.Silu)

            def warmup(n):
                for _ in range(n):
                    jps = psc.tile([128, HB, W], F32, tag="ps", name="jps")
                    flat = jps[:].rearrange("p a b -> p (a b)")
                    nc.tensor.matmul(
                        flat[:, 0:448], jc[:, 0:128], jc[:], start=True, stop=True
                    )

            # ---- routing ----
            pooled = {}  # (s, it, piece) -> [128,1] bf16 partial sums
            r_bcast = {}

            # Two-stage GAP: X-only reduce to [128, rows] keeps the DVE
            # 2x packed mode (scalar-per-partition dst forces 1x), then a
            # tiny second reduce to [128, 1].
            def reduce_dve(s, it, q, rows):
                r0, r1 = rows
                rs = smal.tile(
                    [128, H], BF16, tag=f"rs{s}{it}{q}", name=f"rs{s}{it}{q}"
                )
                p = smal.tile(
                    [128, 1], BF16, tag=f"po{s}{it}{q}", name=f"po{s}{it}{q}"
                )
                with nc.allow_low_precision(reason="GAP partials feed sigmoid"):
                    nc.vector.reduce_sum(
                        rs[:, : r1 - r0],
                        x_sb[s][:, it, r0:r1, :],
                        axis=mybir.AxisListType.X,
                    )
                    nc.vector.reduce_sum(
                        p[:], rs[:, : r1 - r0], axis=mybir.AxisListType.X
                    )
                pooled[s, it, q] = p

            def reduce_dve_half(s, it, q):
                reduce_dve(s, it, q, (q * HHALF, (q + 1) * HHALF))

            def reduce_dve_full(s, it):
                reduce_dve(s, it, 0, (0, H))

            def reduce_act(s, it, q, rows):
                r0, r1 = rows
                p = smal.tile(
                    [128, 1], BF16, tag=f"po{s}{it}{q}", name=f"po{s}{it}{q}"
                )
                sl = x_sb[s][:, it, r0:r1, :]
                with nc.allow_low_precision(reason="GAP partials feed sigmoid"):
                    nc.scalar.activation(sl, sl, AF.Copy, accum_out=p[:])
                pooled[s, it, q] = p

            def reduce_act_half(s, it, q):
                reduce_act(s, it, q, (q * HHALF, (q + 1) * HHALF))

            def reduce_act_full(s, it):
                reduce_act(s, it, 0, (0, H))

            def routing_logits_pe(s):
                lg_ps = pss.tile([1, E], F32, tag="rps", name=f"lgps{s}")
                parts = [k for k in pooled if k[0] == s]
                for i, (s_, it, q) in enumerate(parts):
                    nc.tensor.matmul(
                        lg_ps[:], pooled[s_, it, q][:], rwt_sb[:, it],
                        start=(i == 0), stop=(i == len(parts) - 1),
                    )
                return lg_ps

            def routing_z(s, lg_ps):
                zr = smal.tile([1, E], F32, tag=f"z{s}", name=f"z{s}")
                nc.vector.scalar_tensor_tensor(
                    zr[:], lg_ps[:], 1.0 / PIX, rb_sb, ALU.mult, ALU.add
                )
                return zr

            def routing_bcast(s, zr):
                zb_ps = pss.tile([128, E], F32, tag="rps", name=f"zbps{s}")
                nc.tensor.matmul(zb_ps[:], ones_sb, zr[:], start=True, stop=True)
                return zb_ps

            def routing_sigmoid(s, zb_ps):
                # sigmoid(z) = silu(z) * (1/z)  (z==0 has measure zero;
                # the f32 ratio is numerically stable for small |z|).
                st = smal.tile([128, E], F32, tag=f"st{s}", name=f"st{s}")
                nc.scalar.activation(st[:], zb_ps[:], AF.Silu)
                zi = smal.tile([128, E], F32, tag=f"zi{s}", name=f"zi{s}")
                nc.vector.reciprocal(zi[:], zb_ps[:])
                rbc = smal.tile([128, E], F32, tag=f"rbc{s}", name=f"rbc{s}")
                nc.vector.tensor_tensor(rbc[:], st[:], zi[:], ALU.mult)
                r_bcast[s] = rbc

            cw_r = {
                (s, it): cwp.tile(
                    [128, SLAB], BF16, tag=f"cwr_{s}_{it}", name=f"cwr_{s}_{it}"
                )
                for s in range(SPC)
                for it in range(IT)
            }
            acc_a = cwp.tile([128, SLAB], BF16, tag="acc_a")
            acc_b = cwp.tile([128, SLAB], BF16, tag="acc_b")
            tmp_t = cwp.tile([128, SLAB], BF16, tag="tmp")
            u_a = cwp.tile([128, SLAB], BF16, tag="u_a")
            u_b = cwp.tile([128, SLAB], BF16, tag="u_b")

            def combine_dve(s, it, c0=0, c1=SLAB, assist=None):
                # TS + TT ping-pong chain in slab-pair arrival order over
                # cols [c0:c1).  Odd-e scale copies can run on the Scalar
                # ("act") or GpSimd ("gp") engine in parallel with the
                # DVE adds.
                dst = cw_r[s, it]
                accs = [acc_a, acc_b]
                us = [u_a, u_b]
                cur = None
                for e in range(E):
                    wt_ap = slab(it, e)[:, c0:c1]
                    sc = r_bcast[s][:, e : e + 1]
                    if e == 0:
                        cur = accs[0]
                        nc.vector.tensor_scalar_mul(cur[:, c0:c1], wt_ap, sc)
                        continue
                    if assist and e in assist:
                        u = us[(e // 2) % 2]
                        nc.scalar.activation(u[:, c0:c1], wt_ap, AF.Copy, scale=sc)
                    else:
                        u = tmp_t
                        nc.vector.tensor_scalar_mul(u[:, c0:c1], wt_ap, sc)
                    nxt = dst if e == E - 1 else accs[e % 2]
                    nc.vector.tensor_tensor(
                        nxt[:, c0:c1], cur[:, c0:c1], u[:, c0:c1], ALU.add
                    )
                    cur = nxt

            # PE combines cw(0,0) cols [DG0:SLAB) via diag-scaled matmuls
            # (diag tiles built on GpSimd from the identity in jc) while
            # the DVE chain covers [0:DG0).
            diag_tiles = [
                smal.tile([128, 128], BF16, tag=f"dg{e}", name=f"dg{e}")
                for e in range(E)
            ]

            def diag_builds():
                # On DVE: cheap [128,128] scales that fit in the combine
                # chain's slab-arrival gaps (ACT versions cost ~400ns each
                # and blocked the chain's scale assists).
                for e in range(E):
                    nc.vector.tensor_scalar_mul(
                        diag_tiles[e][:], jc[:, 0:128], r_bcast[0][:, e : e + 1]
                    )

            def combine_pe_diag():
                chunks = [(DG0, DG0 + 448), (DG0 + 448, SLAB)]
                pts = []
                for i, (c0, c1) in enumerate(chunks):
                    t = psc.tile([128, HB, W], F32, tag="ps", name=f"dgps{i}")
                    pts.append(t[:].rearrange("p a b -> p (a b)"))
                for e in range(E):
                    for i, (c0, c1) in enumerate(chunks):
                        nc.tensor.matmul(
                            pts[i][:, : c1 - c0], diag_tiles[e][:],
                            slab(0, e)[:, c0:c1],
                            start=(e == 0), stop=(e == E - 1),
                        )
                for i, (c0, c1) in enumerate(chunks):
                    nc.scalar.activation(
                        cw_r[0, 0][:, c0:c1], pts[i][:, : c1 - c0], AF.Copy
                    )

            hblocks = [(h0, min(HB, H - h0)) for h0 in range(0, H, HB)]
            taps = [(0, 0)] + [
                (dh, dw) for dh in (-1, 0, 1) for dw in (-1, 0, 1) if (dh, dw) != (0, 0)
            ]

            def block_total(h0, nh):
                return IT * sum(
                    1 for dh, dw in taps if min(h0 + nh, H - dh) > max(h0, -dh)
                )

            def conv_block_taps(s, h0, nh, ps_t, its, n_mm, total):
                for it in its:
                    for dh, dw in taps:
                        khkw = (dh + 1) * 3 + (dw + 1)
                        ho_s = max(h0, -dh)
                        ho_e = min(h0 + nh, H - dh)
                        if ho_e <= ho_s:
                            continue
                        nhh = ho_e - ho_s
                        hi_s = ho_s + dh
                        off = khkw * 128
                        lhsT = cw_r[s, it][:, off : off + 128]
                        rhs = x_sb[s][:, it, hi_s : hi_s + nhh, 1 + dw : 1 + dw + W]
                        out = ps_t[:, ho_s - h0 : ho_s - h0 + nhh, 0:W]
                        nc.tensor.matmul(
                            out, lhsT, rhs,
                            start=(n_mm == 0), stop=(n_mm == total - 1),
                        )
                        n_mm += 1
                return n_mm

            def conv_epilogue(s, h0, nh, ps_t):
                o_t = outp.tile([128, HB, W], BF16, tag="out", name="o_t")
                nc.scalar.activation(
                    o_t[:, :nh, :], ps_t[:, :nh, :], AF.Silu,
                    bias=bnb_sb[:], scale=bns_sb[:],
                )
                nc.sync.dma_start(y_d[s, :, h0 : h0 + nh, :], o_t[:, :nh, :])

            def conv_sample_phased(s, weave_a=None, weave_bp=None):
                # weave_bp runs after phase-B block bi's taps but BEFORE
                # its epilogue, so a routing sigmoid emitted there lands
                # ahead of the Silu epilogues on the ACT queue.
                weave_a = weave_a or {}
                weave_bp = weave_bp or {}
                pa_s = []
                for bi, (h0, nh) in enumerate(hblocks):
                    ps_t = psc.tile([128, HB, W], F32, tag="ps", name="ps")
                    total = block_total(h0, nh)
                    n_mm = conv_block_taps(s, h0, nh, ps_t, [0], 0, total)
                    pa_s.append((h0, nh, ps_t, n_mm, total))
                    if bi in weave_a:
                        weave_a[bi]()
                for bi, (h0, nh, ps_t, n_mm, total) in enumerate(pa_s):
                    n_mm = conv_block_taps(s, h0, nh, ps_t, [1], n_mm, total)
                    assert n_mm == total
                    if bi in weave_bp:
                        weave_bp[bi]()
                    conv_epilogue(s, h0, nh, ps_t)

            # ================= emission schedule =================
            # s0 GAP: 6 third-reduces chasing the x0 piece DMAs,
            # alternating DVE / ACT so neither serializes.
            for it in range(IT):
                for qi, (r0, r1) in enumerate(THIRDS):
                    if (it * 3 + qi) % 2 == 0:
                        reduce_dve(0, it, qi, (r0, r1))
                    else:
                        reduce_act(0, it, qi, (r0, r1))

            warmup(JUNK_A)
            lg0 = routing_logits_pe(0)
            z0 = routing_z(0, lg0)
            warmup(JUNK_M)
            zb0 = routing_bcast(0, z0)
            routing_sigmoid(0, zb0)
            warmup(JUNK_B)
            diag_builds()
            combine_pe_diag()

            combine_dve(0, 0, c1=DG0, assist=(1, 3, 5, 7))
            combine_dve(0, 1, assist=(1, 3, 5, 7))
            # s1 GAP: quarter reduces chasing the split x1 DMAs — ACT
            # halves first (free then), DVE halves after chain(0,1).
            reduce_act_half(1, 0, 1)
            reduce_act_half(1, 1, 1)
            reduce_dve_half(1, 0, 0)
            reduce_dve_half(1, 1, 0)

            def weave_routing(s):
                lg = routing_logits_pe(s)
                z = routing_z(s, lg)
                zb = routing_bcast(s, z)
                routing_sigmoid(s, zb)

            def weave_s1():
                # s1 routing + chains; the ACT table never swaps now, so
                # chain(1,0) can take the full odd-expert assist set.
                weave_routing(1)
                combine_dve(1, 0, assist=(1, 3, 5, 7))
                combine_dve(1, 1)

            def weave_s23(s):
                def w():
                    weave_routing(s + 1)
                    combine_dve(s + 1, 0, assist=(1, 3, 5, 7))
                    combine_dve(s + 1, 1)
                return w

            def red4(s):
                # s's GAP as 4 quarter reduces on ACT, woven between
                # epilogues of an earlier phase-B.
                return {
                    2: lambda: reduce_act_half(s, 0, 0),
                    3: lambda: reduce_act_half(s, 0, 1),
                    4: lambda: reduce_act_half(s, 1, 0),
                    5: lambda: reduce_act_half(s, 1, 1),
                }

            conv_sample_phased(0, weave_bp={1: weave_s1, **red4(2)})
            conv_sample_phased(1, weave_a={5: weave_s23(1)}, weave_bp=red4(3))
            conv_sample_phased(2, weave_a={3: weave_s23(2)})
            conv_sample_phased(3)

    nc.compile()
    return nc


def _get_program():
    if "nc" not in _PROGRAM_CACHE:
        _PROGRAM_CACHE["nc"] = _build_program()
    return _PROGRAM_CACHE["nc"]


def kernel(x, routing_w, routing_b, kernel_weights, bn_gamma, bn_beta, bn_mean, bn_var,
           _trace=False, _trace_kwargs=None):
    x = np.asarray(x, dtype=np.float32)
    routing_w = np.asarray(routing_w, dtype=np.float32)
    routing_b = np.asarray(routing_b, dtype=np.float32)
    kernel_weights = np.asarray(kernel_weights, dtype=np.float32)
    bn_gamma = np.asarray(bn_gamma, dtype=np.float32)
    bn_beta = np.asarray(bn_beta, dtype=np.float32)
    bn_mean = np.asarray(bn_mean, dtype=np.float32)
    bn_var = np.asarray(bn_var, dtype=np.float32)

    kwb = kernel_weights.astype(NPBF16)
    kw7 = kwb.reshape(E, OT, 128, IT, 128, KS, KS)
    wt_host = []
    for ot in range(OT):
        # [E, IT, 128cin, KS, KS, 128cout] -> [E, IT, 128, SLAB]
        a = np.ascontiguousarray(kw7[:, ot].transpose(0, 2, 3, 4, 5, 1)).reshape(
            E, IT, 128, SLAB
        )
        # pair-interleave: [IT, NPAIR, 128, 2*SLAB]
        b_ = a.reshape(NPAIR, 2, IT, 128, SLAB).transpose(2, 0, 3, 1, 4)
        wt_host.append(np.ascontiguousarray(b_).reshape(IT, NPAIR, 128, 2 * SLAB))

    rwt_host = np.ascontiguousarray(
        routing_w.T.reshape(IT, 128, E).transpose(1, 0, 2)
    ).astype(NPBF16)
    cst_host = np.zeros((1, 128 + E), dtype=np.float32)
    cst_host[0, :128] = 1.0
    cst_host[0, 128:] = routing_b
    inv = bn_gamma / np.sqrt(bn_var + BN_EPS)
    bnb_full = bn_beta - bn_mean * inv
    bns_host = [
        np.ascontiguousarray(inv[ot * 128 : (ot + 1) * 128]).reshape(128, 1)
        for ot in range(OT)
    ]
    bnb_host = [
        np.ascontiguousarray(bnb_full[ot * 128 : (ot + 1) * 128]).reshape(128, 1)
        for ot in range(OT)
    ]

    jc_host = np.zeros((128, 448), dtype=NPBF16)
    jc_host[:, :128] = np.eye(128, dtype=np.float32).astype(NPBF16)
    jc_host[0, 128:256] = NPBF16(1.0)
    jc_host[:, 256:] = NPBF16(0.25)

    x_pad = np.zeros((B, CIN, H, WP), dtype=NPBF16)
    x_pad[:, :, :, 1 : 1 + W] = x.astype(NPBF16)
    # [B, CIN, H, WP] -> per group [SPC, 128, IT*H*WP] (partition-major)
    x_host = [
        np.ascontiguousarray(
            x_pad[g * SPC : (g + 1) * SPC]
            .reshape(SPC, IT, 128, H, WP)
            .transpose(0, 2, 1, 3, 4)
        ).reshape(SPC, 128, IT * H * WP)
        for g in range(B // SPC)
    ]

    in_maps = []
    for c in range(NCORES):
        ot = c % 2
        g = c // 2
        in_maps.append(
            {
                "jc": jc_host,
                "x": x_host[g],
                "wt": wt_host[ot],
                "rwt": rwt_host,
                "cst": cst_host,
                "bns": bns_host[ot],
                "bnb": bnb_host[ot],
            }
        )

    nc = _get_program()
    res = run_bass_kernel_spmd(
        nc, in_maps, core_ids=list(range(NCORES)),
        trace=_trace, **(_trace_kwargs or {}),
    )
    _PROGRAM_CACHE["last_result"] = res

    out = np.empty((B, COUT, H, W), dtype=np.float32)
    for c in range(NCORES):
        ot = c % 2
        g = c // 2
        yg = res.results[c]["y"]
        out[g * SPC : (g + 1) * SPC, ot * 128 : (ot + 1) * 128] = np.asarray(
            yg
        ).astype(np.float32)
    return out


# revision 56
# speedup vs baseline: 1.0049x; 1.0049x over previous
"""CondConv (MoE routed conv) Trainium2 Bass kernel — v4.

Sharding: core c -> samples [4*(c//2), 4*(c//2)+4), cout half ot = c%2.

Changes vs v3 (135992 ns):
  - All input DMA on the sync ring in strict priority order: gate const,
    x0 quarters, routing smalls, it0 slab pairs, x1, it1 slab pairs,
    x2, x3.  Slabs are loaded as expert PAIRS (host pre-interleaved) so
    the combine chain can chase arrivals; y output DMAs follow on the
    same ring.
  - No gpsimd DMA ring (its swdge-init memsets started the measured
    exec window ~1.1us early); nothing executes before the first
    DIRECT2D.
  - GAP for s0 runs as 4 quarter reduces on DVE with bf16 output
    (2x perf mode), chasing the x0 quarter DMAs; routing runs ~7us
    earlier than v3.
  - s0 combine chains chase the slab-pair arrivals; remaining samples
    use pure-DVE chains (ACT keeps epilogue slack).
  - Routing matmuls for s1..s3 are woven into the conv phase-A stream.
"""

import sys

sys.path.insert(0, "/opt/trn_rl_repo")

import ml_dtypes
import numpy as np

import concourse.bass as bass  # noqa: F401
import concourse.mybir as mybir
import concourse.tile as tile
from concourse import bacc
from concourse.bass_utils import run_bass_kernel_spmd

F32 = mybir.dt.float32
BF16 = mybir.dt.bfloat16
AF = mybir.ActivationFunctionType
ALU = mybir.AluOpType
NPBF16 = ml_dtypes.bfloat16

B, CIN, H, W = 16, 256, 56, 56
E, COUT, KS = 8, 256, 3
NCORES = 8
SPC = 4
IT = CIN // 128
OT = COUT // 128
KHKW = KS * KS
HB = 8
WP = W + 2
PIX = H * W
BN_EPS = 1e-5
SLAB = KHKW * 128  # 1152
NPAIR = E // 2
NPA = 7
JUNK_A = 22
JUNK_M = 2
JUNK_B = 2
HHALF = H // 2
HWP = H * WP

_PROGRAM_CACHE = {}


def _build_program():
    nc = bacc.Bacc("TRN2", target_bir_lowering=False, debug=False)

    jc_d = nc.dram_tensor("jc", [128, 448], BF16, kind="ExternalInput")
    # x is partition-major: each partition's IT*H*WP elements contiguous,
    # so DMA descriptors are one long run per partition (quarters = flat
    # column ranges), not 116-byte rows.
    x_d = nc.dram_tensor("x", [SPC, 128, IT * H * WP], BF16, kind="ExternalInput")
    wt_d = nc.dram_tensor("wt", [IT, NPAIR, 128, 2 * SLAB], BF16, kind="ExternalInput")
    rwt_d = nc.dram_tensor("rwt", [128, IT, E], BF16, kind="ExternalInput")
    # cst cols 0:128 = ones (bcast lhsT), cols 128:128+E = routing bias
    cst_d = nc.dram_tensor("cst", [1, 128 + E], F32, kind="ExternalInput")
    bns_d = nc.dram_tensor("bns", [128, 1], F32, kind="ExternalInput")
    bnb_d = nc.dram_tensor("bnb", [128, 1], F32, kind="ExternalInput")
    y_d = nc.dram_tensor("y", [SPC, 128, H, W], BF16, kind="ExternalOutput")

    with tile.TileContext(nc) as tc:
        with (
            tc.tile_pool(name="xp", bufs=1) as xp,
            tc.tile_pool(name="cwp", bufs=1) as cwp,
            tc.tile_pool(name="wtp", bufs=1) as wtp,
            tc.tile_pool(name="outp", bufs=6) as outp,
            tc.tile_pool(name="smal", bufs=1) as smal,
            tc.tile_pool(name="psc", bufs=NPA, space="PSUM") as psc,
            tc.tile_pool(name="pss", bufs=1, space="PSUM") as pss,
        ):
            # ---- priority-ordered input DMA, all on the sync ring ----
            jc = smal.tile([128, 448], BF16, tag="jc")
            nc.sync.dma_start(jc[:], jc_d[:])

            x_sb = {}
            x_flat = {}
            for s in range(SPC):
                x_sb[s] = xp.tile([128, IT, H, WP], BF16, tag=f"x{s}", name=f"x{s}")
                x_flat[s] = x_sb[s][:].rearrange("p a b c -> p (a b c)")

            QF = HHALF * WP  # flat elems per (it, H-half) quarter
            # x0 row-thirds per it (6 pieces) for a finer GAP chase
            THIRDS = [(0, 18), (18, 37), (37, H)]

            def load_x_rows(s, it, r0, r1):
                c0 = it * HWP + r0 * WP
                c1 = it * HWP + r1 * WP
                nc.sync.dma_start(x_flat[s][:, c0:c1], x_d[s, :, c0:c1])

            def load_x_quarter(s, it, q):
                load_x_rows(s, it, q * HHALF, (q + 1) * HHALF)

            for it in range(IT):
                for r0, r1 in THIRDS:
                    load_x_rows(0, it, r0, r1)

            rwt_sb = smal.tile([128, IT, E], BF16, tag="rwt")
            nc.sync.dma_start(rwt_sb[:], rwt_d[:])
            cst_sb = smal.tile([1, 128 + E], F32, tag="cst")
            nc.sync.dma_start(cst_sb[:], cst_d[:])
            ones_sb = cst_sb[:, 0:128]
            rb_sb = cst_sb[:, 128 : 128 + E]

            pair_tiles = {}
            DG0 = 512

            def load_slab_pairs(it, split=False):
                for p in range(NPAIR):
                    t = wtp.tile(
                        [128, 2 * SLAB], BF16, tag=f"wt{it}{p}", name=f"wt{it}{p}"
                    )
                    if split:
                        # head cols [0:DG0) of both experts first (the DVE
                        # chain region -> no arrival chase), tails after
                        # (the PE diag region chases them).
                        tv = t[:].rearrange("q (e c) -> q e c", e=2)
                        sv = wt_d[it, p].rearrange("q (e c) -> q e c", e=2)
                        nc.sync.dma_start(tv[:, :, 0:DG0], sv[:, :, 0:DG0])
                        nc.sync.dma_start(tv[:, :, DG0:], sv[:, :, DG0:])
                    else:
                        nc.sync.dma_start(t[:], wt_d[it, p])
                    pair_tiles[it, p] = t

            def slab(it, e):
                t = pair_tiles[it, e // 2]
                off = (e % 2) * SLAB
                return t[:, off : off + SLAB]

            load_slab_pairs(0, split=True)
            load_slab_pairs(1)
            bns_sb = smal.tile([128, 1], F32, tag="bns")
            nc.sync.dma_start(bns_sb[:], bns_d[:])
            bnb_sb = smal.tile([128, 1], F32, tag="bnb")
            nc.sync.dma_start(bnb_sb[:], bnb_d[:])
            # x1 split in quarters so its GAP reduces can chase
            for it in range(IT):
                for q in range(2):
                    load_x_quarter(1, it, q)
            nc.sync.dma_start(x_flat[2][:], x_d[2])
            nc.sync.dma_start(x_flat[3][:], x_d[3])

            # ---- ACT table preload (gated on jc, not t=0). Silu is the
            # ONLY table the kernel ever uses: routing computes
            # sigmoid(z) as silu(z)/z so the ACT engine never swaps
            # tables (a swap costs 1.28us and landed on the routing
            # critical path).
            sig_dummy = smal.tile([1, 8], BF16, tag="sigd")
            nc.scalar.activation(sig_dummy[:], jc[0:1, 0:8], AF.Silu)

            def warmup(n):
                for _ in range(n):
                    jps = psc.tile([128, HB, W], F32, tag="ps", name="jps")
                    flat = jps[:].rearrange("p a b -> p (a b)")
                    nc.tensor.matmul(
                        flat[:, 0:448], jc[:, 0:128], jc[:], start=True, stop=True
                    )

            # ---- routing ----
            pooled = {}  # (s, it, piece) -> [128,1] bf16 partial sums
            r_bcast = {}

            # Two-stage GAP: X-only reduce to [128, rows] keeps the DVE
            # 2x packed mode (scalar-per-partition dst forces 1x), then a
            # tiny second reduce to [128, 1].
            def reduce_dve(s, it, q, rows):
                r0, r1 = rows
                rs = smal.tile(
                    [128, H], BF16, tag=f"rs{s}{it}{q}", name=f"rs{s}{it}{q}"
                )
                p = smal.tile(
                    [128, 1], BF16, tag=f"po{s}{it}{q}", name=f"po{s}{it}{q}"
                )
                with nc.allow_low_precision(reason="GAP partials feed sigmoid"):
                    nc.vector.reduce_sum(
                        rs[:, : r1 - r0],
                        x_sb[s][:, it, r0:r1, :],
                        axis=mybir.AxisListType.X,
                    )
                    nc.vector.reduce_sum(
                        p[:], rs[:, : r1 - r0], axis=mybir.AxisListType.X
                    )
                pooled[s, it, q] = p

            def reduce_dve_half(s, it, q):
                reduce_dve(s, it, q, (q * HHALF, (q + 1) * HHALF))

            def reduce_dve_full(s, it):
                reduce_dve(s, it, 0, (0, H))

            def reduce_act(s, it, q, rows):
                r0, r1 = rows
                p = smal.tile(
                    [128, 1], BF16, tag=f"po{s}{it}{q}", name=f"po{s}{it}{q}"
                )
                sl = x_sb[s][:, it, r0:r1, :]
                with nc.allow_low_precision(reason="GAP partials feed sigmoid"):
                    nc.scalar.activation(sl, sl, AF.Copy, accum_out=p[:])
                pooled[s, it, q] = p

            def reduce_act_half(s, it, q):
                reduce_act(s, it, q, (q * HHALF, (q + 1) * HHALF))

            def reduce_act_full(s, it):
                reduce_act(s, it, 0, (0, H))

            def routing_logits_pe(s):
                lg_ps = pss.tile([1, E], F32, tag="rps", name=f"lgps{s}")
                parts = [k for k in pooled if k[0] == s]
                for i, (s_, it, q) in enumerate(parts):
                    nc.tensor.matmul(
                        lg_ps[:], pooled[s_, it, q][:], rwt_sb[:, it],
                        start=(i == 0), stop=(i == len(parts) - 1),
                    )
                return lg_ps

            def routing_z(s, lg_ps):
                zr = smal.tile([1, E], F32, tag=f"z{s}", name=f"z{s}")
                nc.vector.scalar_tensor_tensor(
                    zr[:], lg_ps[:], 1.0 / PIX, rb_sb, ALU.mult, ALU.add
                )
                return zr

            def routing_bcast(s, zr):
                zb_ps = pss.tile([128, E], F32, tag="rps", name=f"zbps{s}")
                nc.tensor.matmul(zb_ps[:], ones_sb, zr[:], start=True, stop=True)
                return zb_ps

            def routing_sigmoid(s, zb_ps):
                # sigmoid(z) = silu(z) * (1/z)  (z==0 has measure zero;
                # the f32 ratio is numerically stable for small |z|).
                st = smal.tile([128, E], F32, tag=f"st{s}", name=f"st{s}")
                nc.scalar.activation(st[:], zb_ps[:], AF.Silu)
                zi = smal.tile([128, E], F32, tag=f"zi{s}", name=f"zi{s}")
                nc.vector.reciprocal(zi[:], zb_ps[:])
                rbc = smal.tile([128, E], F32, tag=f"rbc{s}", name=f"rbc{s}")
                nc.vector.tensor_tensor(rbc[:], st[:], zi[:], ALU.mult)
                r_bcast[s] = rbc

            cw_r = {
                (s, it): cwp.tile(
                    [128, SLAB], BF16, tag=f"cwr_{s}_{it}", name=f"cwr_{s}_{it}"
                )
                for s in range(SPC)
                for it in range(IT)
            }
            acc_a = cwp.tile([128, SLAB], BF16, tag="acc_a")
            acc_b = cwp.tile([128, SLAB], BF16, tag="acc_b")
            tmp_t = cwp.tile([128, SLAB], BF16, tag="tmp")
            u_a = cwp.tile([128, SLAB], BF16, tag="u_a")
            u_b = cwp.tile([128, SLAB], BF16, tag="u_b")

            def combine_dve(s, it, c0=0, c1=SLAB, assist=None):
                # TS + TT ping-pong chain in slab-pair arrival order over
                # cols [c0:c1).  Odd-e scale copies can run on the Scalar
                # ("act") or GpSimd ("gp") engine in parallel with the
                # DVE adds.
                dst = cw_r[s, it]
                accs = [acc_a, acc_b]
                us = [u_a, u_b]
                cur = None
                for e in range(E):
                    wt_ap = slab(it, e)[:, c0:c1]
                    sc = r_bcast[s][:, e : e + 1]
                    if e == 0:
                        cur = accs[0]
                        nc.vector.tensor_scalar_mul(cur[:, c0:c1], wt_ap, sc)
                        continue
                    if assist and e in assist:
                        u = us[(e // 2) % 2]
                        nc.scalar.activation(u[:, c0:c1], wt_ap, AF.Copy, scale=sc)
                    else:
                        u = tmp_t
                        nc.vector.tensor_scalar_mul(u[:, c0:c1], wt_ap, sc)
                    nxt = dst if e == E - 1 else accs[e % 2]
                    nc.vector.tensor_tensor(
                        nxt[:, c0:c1], cur[:, c0:c1], u[:, c0:c1], ALU.add
                    )
                    cur = nxt

            # PE combines cw(0,0) cols [DG0:SLAB) via diag-scaled matmuls
            # (diag tiles built on GpSimd from the identity in jc) while
            # the DVE chain covers [0:DG0).
            diag_tiles = [
                smal.tile([128, 128], BF16, tag=f"dg{e}", name=f"dg{e}")
                for e in range(E)
            ]

            def diag_builds():
                # On DVE: cheap [128,128] scales that fit in the combine
                # chain's slab-arrival gaps (ACT versions cost ~400ns each
                # and blocked the chain's scale assists).
                for e in range(E):
                    nc.vector.tensor_scalar_mul(
                        diag_tiles[e][:], jc[:, 0:128], r_bcast[0][:, e : e + 1]
                    )

            def combine_pe_diag():
                chunks = [(DG0, DG0 + 448), (DG0 + 448, SLAB)]
                pts = []
                for i, (c0, c1) in enumerate(chunks):
                    t = psc.tile([128, HB, W], F32, tag="ps", name=f"dgps{i}")
                    pts.append(t[:].rearrange("p a b -> p (a b)"))
                for e in range(E):
                    for i, (c0, c1) in enumerate(chunks):
                        nc.tensor.matmul(
                            pts[i][:, : c1 - c0], diag_tiles[e][:],
                            slab(0, e)[:, c0:c1],
                            start=(e == 0), stop=(e == E - 1),
                        )
                for i, (c0, c1) in enumerate(chunks):
                    nc.scalar.activation(
                        cw_r[0, 0][:, c0:c1], pts[i][:, : c1 - c0], AF.Copy
                    )

            hblocks = [(h0, min(HB, H - h0)) for h0 in range(0, H, HB)]
            taps = [(0, 0)] + [
                (dh, dw) for dh in (-1, 0, 1) for dw in (-1, 0, 1) if (dh, dw) != (0, 0)
            ]

            def block_total(h0, nh):
                return IT * sum(
                    1 for dh, dw in taps if min(h0 + nh, H - dh) > max(h0, -dh)
                )

            def conv_block_taps(s, h0, nh, ps_t, its, n_mm, total):
                for it in its:
                    for dh, dw in taps:
                        khkw = (dh + 1) * 3 + (dw + 1)
                        ho_s = max(h0, -dh)
                        ho_e = min(h0 + nh, H - dh)
                        if ho_e <= ho_s:
                            continue
                        nhh = ho_e - ho_s
                        hi_s = ho_s + dh
                        off = khkw * 128
                        lhsT = cw_r[s, it][:, off : off + 128]
                        rhs = x_sb[s][:, it, hi_s : hi_s + nhh, 1 + dw : 1 + dw + W]
                        out = ps_t[:, ho_s - h0 : ho_s - h0 + nhh, 0:W]
                        nc.tensor.matmul(
                            out, lhsT, rhs,
                            start=(n_mm == 0), stop=(n_mm == total - 1),
                        )
                        n_mm += 1
                return n_mm

            def conv_epilogue(s, h0, nh, ps_t):
                o_t = outp.tile([128, HB, W], BF16, tag="out", name="o_t")
                nc.scalar.activation(
                    o_t[:, :nh, :], ps_t[:, :nh, :], AF.Silu,
                    bias=bnb_sb[:], scale=bns_sb[:],
                )
                nc.sync.dma_start(y_d[s, :, h0 : h0 + nh, :], o_t[:, :nh, :])

            def conv_sample_phased(s, weave_a=None, weave_bp=None):
                # weave_bp runs after phase-B block bi's taps but BEFORE
                # its epilogue, so a routing sigmoid emitted there lands
                # ahead of the Silu epilogues on the ACT queue.
                weave_a = weave_a or {}
                weave_bp = weave_bp or {}
                pa_s = []
                for bi, (h0, nh) in enumerate(hblocks):
                    ps_t = psc.tile([128, HB, W], F32, tag="ps", name="ps")
                    total = block_total(h0, nh)
                    n_mm = conv_block_taps(s, h0, nh, ps_t, [0], 0, total)
                    pa_s.append((h0, nh, ps_t, n_mm, total))
                    if bi in weave_a:
                        weave_a[bi]()
                for bi, (h0, nh, ps_t, n_mm, total) in enumerate(pa_s):
                    n_mm = conv_block_taps(s, h0, nh, ps_t, [1], n_mm, total)
                    assert n_mm == total
                    if bi in weave_bp:
                        weave_bp[bi]()
                    conv_epilogue(s, h0, nh, ps_t)

            # ================= emission schedule =================
            # s0 GAP: 6 third-reduces chasing the x0 piece DMAs,
            # alternating DVE / ACT so neither serializes.
            for it in range(IT):
                for qi, (r0, r1) in enumerate(THIRDS):
                    if (it * 3 + qi) % 2 == 0:
                        reduce_dve(0, it, qi, (r0, r1))
                    else:
                        reduce_act(0, it, qi, (r0, r1))

            warmup(JUNK_A)
            lg0 = routing_logits_pe(0)
            z0 = routing_z(0, lg0)
            warmup(JUNK_M)
            zb0 = routing_bcast(0, z0)
            routing_sigmoid(0, zb0)
            warmup(JUNK_B)
            diag_builds()
            combine_pe_diag()

            combine_dve(0, 0, c1=DG0, assist=(1, 3, 5, 7))
            combine_dve(0, 1, assist=(1, 3, 5, 7))
            # s1 GAP: quarter reduces chasing the split x1 DMAs — ACT
            # halves first (free then), DVE halves after chain(0,1).
            reduce_act_half(1, 0, 1)
            reduce_act_half(1, 1, 1)
            reduce_dve_half(1, 0, 0)
            reduce_dve_half(1, 1, 0)

            def weave_routing(s):
                lg = routing_logits_pe(s)
                z = routing_z(s, lg)
                zb = routing_bcast(s, z)
                routing_sigmoid(s, zb)

            def weave_s1():
                # s1 routing + chains; the ACT table never swaps now, so
                # chain(1,0) can take the full odd-expert assist set.
                weave_routing(1)
                combine_dve(1, 0, assist=(1, 3, 5, 7))
                combine_dve(1, 1)

            def weave_s23(s):
                def w():
                    weave_routing(s + 1)
                    combine_dve(s + 1, 0, assist=(1, 3, 5, 7))
                    combine_dve(s + 1, 1)
                return w

            def red4(s):
                # s's GAP as 4 quarter reduces on ACT, woven between
                # epilogues of an earlier phase-B.
                return {
                    2: lambda: reduce_act_half(s, 0, 0),
                    3: lambda: reduce_act_half(s, 0, 1),
                    4: lambda: reduce_act_half(s, 1, 0),
                    5: lambda: reduce_act_half(s, 1, 1),
                }

            conv_sample_phased(0, weave_bp={1: weave_s1, **red4(2)})
            conv_sample_phased(1, weave_a={5: weave_s23(1)}, weave_bp=red4(3))
            conv_sample_phased(2, weave_a={3: weave_s23(2)})
            conv_sample_phased(3)

    nc.compile()
    return nc


def _get_program():
    if "nc" not in _PROGRAM_CACHE:
        _PROGRAM_CACHE["nc"] = _build_program()
    return _PROGRAM_CACHE["nc"]


def kernel(x, routing_w, routing_b, kernel_weights, bn_gamma, bn_beta, bn_mean, bn_var,
           _trace=False, _trace_kwargs=None):
    x = np.asarray(x, dtype=np.float32)
    routing_w = np.asarray(routing_w, dtype=np.float32)
    routing_b = np.asarray(routing_b, dtype=np.float32)
    kernel_weights = np.asarray(kernel_weights, dtype=np.float32)
    bn_gamma = np.asarray(bn_gamma, dtype=np.float32)
    bn_beta = np.asarray(bn_beta, dtype=np.float32)
    bn_mean = np.asarray(bn_mean, dtype=np.float32)
    bn_var = np.asarray(bn_var, dtype=np.float32)

    kwb = kernel_weights.astype(NPBF16)
    kw7 = kwb.reshape(E, OT, 128, IT, 128, KS, KS)
    wt_host = []
    for ot in range(OT):
        # [E, IT, 128cin, KS, KS, 128cout] -> [E, IT, 128, SLAB]
        a = np.ascontiguousarray(kw7[:, ot].transpose(0, 2, 3, 4, 5, 1)).reshape(
            E, IT, 128, SLAB
        )
        # pair-interleave: [IT, NPAIR, 128, 2*SLAB]
        b_ = a.reshape(NPAIR, 2, IT, 128, SLAB).transpose(2, 0, 3, 1, 4)
        wt_host.append(np.ascontiguousarray(b_).reshape(IT, NPAIR, 128, 2 * SLAB))

    rwt_host = np.ascontiguousarray(
        routing_w.T.reshape(IT, 128, E).transpose(1, 0, 2)
    ).astype(NPBF16)
    cst_host = np.zeros((1, 128 + E), dtype=np.float32)
    cst_host[0, :128] = 1.0
    cst_host[0, 128:] = routing_b
    inv = bn_gamma / np.sqrt(bn_var + BN_EPS)
    bnb_full = bn_beta - bn_mean * inv
    bns_host = [
        np.ascontiguousarray(inv[ot * 128 : (ot + 1) * 128]).reshape(128, 1)
        for ot in range(OT)
    ]
    bnb_host = [
        np.ascontiguousarray(bnb_full[ot * 128 : (ot + 1) * 128]).reshape(128, 1)
        for ot in range(OT)
    ]

    jc_host = np.zeros((128, 448), dtype=NPBF16)
    jc_host[:, :128] = np.eye(128, dtype=np.float32).astype(NPBF16)
    jc_host[0, 128:256] = NPBF16(1.0)
    jc_host[:, 256:] = NPBF16(0.25)

    x_pad = np.zeros((B, CIN, H, WP), dtype=NPBF16)
    x_pad[:, :, :, 1 : 1 + W] = x.astype(NPBF16)
    # [B, CIN, H, WP] -> per group [SPC, 128, IT*H*WP] (partition-major)
    x_host = [
        np.ascontiguousarray(
            x_pad[g * SPC : (g + 1) * SPC]
            .reshape(SPC, IT, 128, H, WP)
            .transpose(0, 2, 1, 3, 4)
        ).reshape(SPC, 128, IT * H * WP)
        for g in range(B // SPC)
    ]

    in_maps = []
    for c in range(NCORES):
        ot = c % 2
        g = c // 2
        in_maps.append(
            {
                "jc": jc_host,
                "x": x_host[g],
                "wt": wt_host[ot],
                "rwt": rwt_host,
                "cst": cst_host,
                "bns": bns_host[ot],
                "bnb": bnb_host[ot],
            }
        )

    nc = _get_program()
    res = run_bass_kernel_spmd(
        nc, in_maps, core_ids=list(range(NCORES)),
        trace=_trace, **(_trace_kwargs or {}),
    )
    _PROGRAM_CACHE["last_result"] = res

    out = np.empty((B, COUT, H, W), dtype=np.float32)
    for c in range(NCORES):
        ot = c % 2
        g = c // 2
        yg = res.results[c]["y"]
        out[g * SPC : (g + 1) * SPC, ot * 128 : (ot + 1) * 128] = np.asarray(
            yg
        ).astype(np.float32)
    return out


# revision 57
# speedup vs baseline: 1.0086x; 1.0037x over previous
"""CondConv (MoE routed conv) Trainium2 Bass kernel — v4.

Sharding: core c -> samples [4*(c//2), 4*(c//2)+4), cout half ot = c%2.

Changes vs v3 (135992 ns):
  - All input DMA on the sync ring in strict priority order: gate const,
    x0 quarters, routing smalls, it0 slab pairs, x1, it1 slab pairs,
    x2, x3.  Slabs are loaded as expert PAIRS (host pre-interleaved) so
    the combine chain can chase arrivals; y output DMAs follow on the
    same ring.
  - No gpsimd DMA ring (its swdge-init memsets started the measured
    exec window ~1.1us early); nothing executes before the first
    DIRECT2D.
  - GAP for s0 runs as 4 quarter reduces on DVE with bf16 output
    (2x perf mode), chasing the x0 quarter DMAs; routing runs ~7us
    earlier than v3.
  - s0 combine chains chase the slab-pair arrivals; remaining samples
    use pure-DVE chains (ACT keeps epilogue slack).
  - Routing matmuls for s1..s3 are woven into the conv phase-A stream.
"""

import sys

sys.path.insert(0, "/opt/trn_rl_repo")

import ml_dtypes
import numpy as np

import concourse.bass as bass  # noqa: F401
import concourse.mybir as mybir
import concourse.tile as tile
from concourse import bacc
from concourse.bass_utils import run_bass_kernel_spmd

F32 = mybir.dt.float32
BF16 = mybir.dt.bfloat16
AF = mybir.ActivationFunctionType
ALU = mybir.AluOpType
NPBF16 = ml_dtypes.bfloat16

B, CIN, H, W = 16, 256, 56, 56
E, COUT, KS = 8, 256, 3
NCORES = 8
SPC = 4
IT = CIN // 128
OT = COUT // 128
KHKW = KS * KS
HB = 8
WP = W + 2
PIX = H * W
BN_EPS = 1e-5
SLAB = KHKW * 128  # 1152
NPAIR = E // 2
NPA = 7
JUNK_A = 22
JUNK_M = 2
JUNK_B = 2
HHALF = H // 2
HWP = H * WP

_PROGRAM_CACHE = {}


def _build_program():
    nc = bacc.Bacc("TRN2", target_bir_lowering=False, debug=False)

    jc_d = nc.dram_tensor("jc", [128, 448], BF16, kind="ExternalInput")
    # x is partition-major: each partition's IT*H*WP elements contiguous,
    # so DMA descriptors are one long run per partition (quarters = flat
    # column ranges), not 116-byte rows.
    x_d = nc.dram_tensor("x", [SPC, 128, IT * H * WP], BF16, kind="ExternalInput")
    wt_d = nc.dram_tensor("wt", [IT, NPAIR, 128, 2 * SLAB], BF16, kind="ExternalInput")
    rwt_d = nc.dram_tensor("rwt", [128, IT, E], BF16, kind="ExternalInput")
    # cst cols 0:128 = ones (bcast lhsT), cols 128:128+E = routing bias
    cst_d = nc.dram_tensor("cst", [1, 128 + E], F32, kind="ExternalInput")
    bns_d = nc.dram_tensor("bns", [128, 1], F32, kind="ExternalInput")
    bnb_d = nc.dram_tensor("bnb", [128, 1], F32, kind="ExternalInput")
    y_d = nc.dram_tensor("y", [SPC, 128, H, W], BF16, kind="ExternalOutput")

    with tile.TileContext(nc) as tc:
        with (
            tc.tile_pool(name="xp", bufs=1) as xp,
            tc.tile_pool(name="cwp", bufs=1) as cwp,
            tc.tile_pool(name="wtp", bufs=1) as wtp,
            tc.tile_pool(name="outp", bufs=6) as outp,
            tc.tile_pool(name="smal", bufs=1) as smal,
            tc.tile_pool(name="psc", bufs=NPA, space="PSUM") as psc,
            tc.tile_pool(name="pss", bufs=1, space="PSUM") as pss,
        ):
            # ---- priority-ordered input DMA, all on the sync ring ----
            jc = smal.tile([128, 448], BF16, tag="jc")
            nc.sync.dma_start(jc[:], jc_d[:])

            x_sb = {}
            x_flat = {}
            for s in range(SPC):
                x_sb[s] = xp.tile([128, IT, H, WP], BF16, tag=f"x{s}", name=f"x{s}")
                x_flat[s] = x_sb[s][:].rearrange("p a b c -> p (a b c)")

            QF = HHALF * WP  # flat elems per (it, H-half) quarter
            # x0 row-thirds per it (6 pieces) for a finer GAP chase
            THIRDS = [(0, 18), (18, 37), (37, H)]

            def load_x_rows(s, it, r0, r1):
                c0 = it * HWP + r0 * WP
                c1 = it * HWP + r1 * WP
                nc.sync.dma_start(x_flat[s][:, c0:c1], x_d[s, :, c0:c1])

            def load_x_quarter(s, it, q):
                load_x_rows(s, it, q * HHALF, (q + 1) * HHALF)

            for it in range(IT):
                for q in range(2):
                    load_x_quarter(0, it, q)

            rwt_sb = smal.tile([128, IT, E], BF16, tag="rwt")
            nc.sync.dma_start(rwt_sb[:], rwt_d[:])
            cst_sb = smal.tile([1, 128 + E], F32, tag="cst")
            nc.sync.dma_start(cst_sb[:], cst_d[:])
            ones_sb = cst_sb[:, 0:128]
            rb_sb = cst_sb[:, 128 : 128 + E]

            pair_tiles = {}
            DG0 = 512

            def load_slab_pairs(it, split=False):
                for p in range(NPAIR):
                    t = wtp.tile(
                        [128, 2 * SLAB], BF16, tag=f"wt{it}{p}", name=f"wt{it}{p}"
                    )
                    if split:
                        # head cols [0:DG0) of both experts first (the DVE
                        # chain region -> no arrival chase), tails after
                        # (the PE diag region chases them).
                        tv = t[:].rearrange("q (e c) -> q e c", e=2)
                        sv = wt_d[it, p].rearrange("q (e c) -> q e c", e=2)
                        nc.sync.dma_start(tv[:, :, 0:DG0], sv[:, :, 0:DG0])
                        nc.sync.dma_start(tv[:, :, DG0:], sv[:, :, DG0:])
                    else:
                        nc.sync.dma_start(t[:], wt_d[it, p])
                    pair_tiles[it, p] = t

            def slab(it, e):
                t = pair_tiles[it, e // 2]
                off = (e % 2) * SLAB
                return t[:, off : off + SLAB]

            load_slab_pairs(0)
            load_slab_pairs(1)
            bns_sb = smal.tile([128, 1], F32, tag="bns")
            nc.sync.dma_start(bns_sb[:], bns_d[:])
            bnb_sb = smal.tile([128, 1], F32, tag="bnb")
            nc.sync.dma_start(bnb_sb[:], bnb_d[:])
            # x1 split in quarters so its GAP reduces can chase
            for it in range(IT):
                for q in range(2):
                    load_x_quarter(1, it, q)
            nc.sync.dma_start(x_flat[2][:], x_d[2])
            nc.sync.dma_start(x_flat[3][:], x_d[3])

            # ---- ACT table preload (gated on jc, not t=0). Silu is the
            # ONLY table the kernel ever uses: routing computes
            # sigmoid(z) as silu(z)/z so the ACT engine never swaps
            # tables (a swap costs 1.28us and landed on the routing
            # critical path).
            sig_dummy = smal.tile([1, 8], BF16, tag="sigd")
            nc.scalar.activation(sig_dummy[:], jc[0:1, 0:8], AF.Silu)

            def warmup(n):
                for _ in range(n):
                    jps = psc.tile([128, HB, W], F32, tag="ps", name="jps")
                    flat = jps[:].rearrange("p a b -> p (a b)")
                    nc.tensor.matmul(
                        flat[:, 0:448], jc[:, 0:128], jc[:], start=True, stop=True
                    )

            # ---- routing ----
            pooled = {}  # (s, it, piece) -> [128,1] bf16 partial sums
            r_bcast = {}

            # Two-stage GAP: X-only reduce to [128, rows] keeps the DVE
            # 2x packed mode (scalar-per-partition dst forces 1x), then a
            # tiny second reduce to [128, 1].
            def reduce_dve(s, it, q, rows):
                r0, r1 = rows
                rs = smal.tile(
                    [128, H], BF16, tag=f"rs{s}{it}{q}", name=f"rs{s}{it}{q}"
                )
                p = smal.tile(
                    [128, 1], BF16, tag=f"po{s}{it}{q}", name=f"po{s}{it}{q}"
                )
                with nc.allow_low_precision(reason="GAP partials feed sigmoid"):
                    nc.vector.reduce_sum(
                        rs[:, : r1 - r0],
                        x_sb[s][:, it, r0:r1, :],
                        axis=mybir.AxisListType.X,
                    )
                    nc.vector.reduce_sum(
                        p[:], rs[:, : r1 - r0], axis=mybir.AxisListType.X
                    )
                pooled[s, it, q] = p

            def reduce_dve_half(s, it, q):
                reduce_dve(s, it, q, (q * HHALF, (q + 1) * HHALF))

            def reduce_dve_full(s, it):
                reduce_dve(s, it, 0, (0, H))

            def reduce_act(s, it, q, rows):
                r0, r1 = rows
                p = smal.tile(
                    [128, 1], BF16, tag=f"po{s}{it}{q}", name=f"po{s}{it}{q}"
                )
                sl = x_sb[s][:, it, r0:r1, :]
                with nc.allow_low_precision(reason="GAP partials feed sigmoid"):
                    nc.scalar.activation(sl, sl, AF.Copy, accum_out=p[:])
                pooled[s, it, q] = p

            def reduce_act_half(s, it, q):
                reduce_act(s, it, q, (q * HHALF, (q + 1) * HHALF))

            def reduce_act_full(s, it):
                reduce_act(s, it, 0, (0, H))

            def routing_logits_pe(s):
                lg_ps = pss.tile([1, E], F32, tag="rps", name=f"lgps{s}")
                parts = [k for k in pooled if k[0] == s]
                for i, (s_, it, q) in enumerate(parts):
                    nc.tensor.matmul(
                        lg_ps[:], pooled[s_, it, q][:], rwt_sb[:, it],
                        start=(i == 0), stop=(i == len(parts) - 1),
                    )
                return lg_ps

            def routing_z(s, lg_ps):
                zr = smal.tile([1, E], F32, tag=f"z{s}", name=f"z{s}")
                nc.vector.scalar_tensor_tensor(
                    zr[:], lg_ps[:], 1.0 / PIX, rb_sb, ALU.mult, ALU.add
                )
                return zr

            def routing_bcast(s, zr):
                zb_ps = pss.tile([128, E], F32, tag="rps", name=f"zbps{s}")
                nc.tensor.matmul(zb_ps[:], ones_sb, zr[:], start=True, stop=True)
                return zb_ps

            def routing_sigmoid(s, zb_ps):
                # sigmoid(z) = silu(z) * (1/z)  (z==0 has measure zero;
                # the f32 ratio is numerically stable for small |z|).
                st = smal.tile([128, E], F32, tag=f"st{s}", name=f"st{s}")
                nc.scalar.activation(st[:], zb_ps[:], AF.Silu)
                zi = smal.tile([128, E], F32, tag=f"zi{s}", name=f"zi{s}")
                nc.vector.reciprocal(zi[:], zb_ps[:])
                rbc = smal.tile([128, E], F32, tag=f"rbc{s}", name=f"rbc{s}")
                nc.vector.tensor_tensor(rbc[:], st[:], zi[:], ALU.mult)
                r_bcast[s] = rbc

            cw_r = {
                (s, it): cwp.tile(
                    [128, SLAB], BF16, tag=f"cwr_{s}_{it}", name=f"cwr_{s}_{it}"
                )
                for s in range(SPC)
                for it in range(IT)
            }
            acc_a = cwp.tile([128, SLAB], BF16, tag="acc_a")
            acc_b = cwp.tile([128, SLAB], BF16, tag="acc_b")
            tmp_t = cwp.tile([128, SLAB], BF16, tag="tmp")
            u_a = cwp.tile([128, SLAB], BF16, tag="u_a")
            u_b = cwp.tile([128, SLAB], BF16, tag="u_b")

            def combine_dve(s, it, c0=0, c1=SLAB, assist=None):
                # TS + TT ping-pong chain in slab-pair arrival order over
                # cols [c0:c1).  Odd-e scale copies can run on the Scalar
                # ("act") or GpSimd ("gp") engine in parallel with the
                # DVE adds.
                dst = cw_r[s, it]
                accs = [acc_a, acc_b]
                us = [u_a, u_b]
                cur = None
                for e in range(E):
                    wt_ap = slab(it, e)[:, c0:c1]
                    sc = r_bcast[s][:, e : e + 1]
                    if e == 0:
                        cur = accs[0]
                        nc.vector.tensor_scalar_mul(cur[:, c0:c1], wt_ap, sc)
                        continue
                    if assist and e in assist:
                        u = us[(e // 2) % 2]
                        nc.scalar.activation(u[:, c0:c1], wt_ap, AF.Copy, scale=sc)
                    else:
                        u = tmp_t
                        nc.vector.tensor_scalar_mul(u[:, c0:c1], wt_ap, sc)
                    nxt = dst if e == E - 1 else accs[e % 2]
                    nc.vector.tensor_tensor(
                        nxt[:, c0:c1], cur[:, c0:c1], u[:, c0:c1], ALU.add
                    )
                    cur = nxt

            # PE combines cw(0,0) cols [DG0:SLAB) via diag-scaled matmuls
            # (diag tiles built on GpSimd from the identity in jc) while
            # the DVE chain covers [0:DG0).
            diag_tiles = [
                smal.tile([128, 128], BF16, tag=f"dg{e}", name=f"dg{e}")
                for e in range(E)
            ]

            def diag_builds():
                # On DVE: cheap [128,128] scales that fit in the combine
                # chain's slab-arrival gaps (ACT versions cost ~400ns each
                # and blocked the chain's scale assists).
                for e in range(E):
                    nc.vector.tensor_scalar_mul(
                        diag_tiles[e][:], jc[:, 0:128], r_bcast[0][:, e : e + 1]
                    )

            def combine_pe_diag():
                chunks = [(DG0, DG0 + 448), (DG0 + 448, SLAB)]
                pts = []
                for i, (c0, c1) in enumerate(chunks):
                    t = psc.tile([128, HB, W], F32, tag="ps", name=f"dgps{i}")
                    pts.append(t[:].rearrange("p a b -> p (a b)"))
                for e in range(E):
                    for i, (c0, c1) in enumerate(chunks):
                        nc.tensor.matmul(
                            pts[i][:, : c1 - c0], diag_tiles[e][:],
                            slab(0, e)[:, c0:c1],
                            start=(e == 0), stop=(e == E - 1),
                        )
                for i, (c0, c1) in enumerate(chunks):
                    nc.scalar.activation(
                        cw_r[0, 0][:, c0:c1], pts[i][:, : c1 - c0], AF.Copy
                    )

            hblocks = [(h0, min(HB, H - h0)) for h0 in range(0, H, HB)]
            taps = [(0, 0)] + [
                (dh, dw) for dh in (-1, 0, 1) for dw in (-1, 0, 1) if (dh, dw) != (0, 0)
            ]

            def block_total(h0, nh):
                return IT * sum(
                    1 for dh, dw in taps if min(h0 + nh, H - dh) > max(h0, -dh)
                )

            def conv_block_taps(s, h0, nh, ps_t, its, n_mm, total):
                for it in its:
                    for dh, dw in taps:
                        khkw = (dh + 1) * 3 + (dw + 1)
                        ho_s = max(h0, -dh)
                        ho_e = min(h0 + nh, H - dh)
                        if ho_e <= ho_s:
                            continue
                        nhh = ho_e - ho_s
                        hi_s = ho_s + dh
                        off = khkw * 128
                        lhsT = cw_r[s, it][:, off : off + 128]
                        rhs = x_sb[s][:, it, hi_s : hi_s + nhh, 1 + dw : 1 + dw + W]
                        out = ps_t[:, ho_s - h0 : ho_s - h0 + nhh, 0:W]
                        nc.tensor.matmul(
                            out, lhsT, rhs,
                            start=(n_mm == 0), stop=(n_mm == total - 1),
                        )
                        n_mm += 1
                return n_mm

            def conv_epilogue(s, h0, nh, ps_t):
                o_t = outp.tile([128, HB, W], BF16, tag="out", name="o_t")
                nc.scalar.activation(
                    o_t[:, :nh, :], ps_t[:, :nh, :], AF.Silu,
                    bias=bnb_sb[:], scale=bns_sb[:],
                )
                nc.sync.dma_start(y_d[s, :, h0 : h0 + nh, :], o_t[:, :nh, :])

            def conv_sample_phased(s, weave_a=None, weave_bp=None):
                # weave_bp runs after phase-B block bi's taps but BEFORE
                # its epilogue, so a routing sigmoid emitted there lands
                # ahead of the Silu epilogues on the ACT queue.
                weave_a = weave_a or {}
                weave_bp = weave_bp or {}
                pa_s = []
                for bi, (h0, nh) in enumerate(hblocks):
                    ps_t = psc.tile([128, HB, W], F32, tag="ps", name="ps")
                    total = block_total(h0, nh)
                    n_mm = conv_block_taps(s, h0, nh, ps_t, [0], 0, total)
                    pa_s.append((h0, nh, ps_t, n_mm, total))
                    if bi in weave_a:
                        weave_a[bi]()
                for bi, (h0, nh, ps_t, n_mm, total) in enumerate(pa_s):
                    n_mm = conv_block_taps(s, h0, nh, ps_t, [1], n_mm, total)
                    assert n_mm == total
                    if bi in weave_bp:
                        weave_bp[bi]()
                    conv_epilogue(s, h0, nh, ps_t)

            # ================= emission schedule =================
            # s0 GAP: 4 quarter reduces chasing the x0 quarter DMAs,
            # alternating DVE / ACT so neither serializes.
            reduce_dve_half(0, 0, 0)
            reduce_act_half(0, 0, 1)
            reduce_dve_half(0, 1, 0)
            reduce_act_half(0, 1, 1)

            warmup(JUNK_A)
            lg0 = routing_logits_pe(0)
            z0 = routing_z(0, lg0)
            warmup(JUNK_M)
            zb0 = routing_bcast(0, z0)
            routing_sigmoid(0, zb0)
            warmup(JUNK_B)
            diag_builds()
            combine_pe_diag()

            combine_dve(0, 0, c1=DG0, assist=(1, 3, 5, 7))
            combine_dve(0, 1, assist=(1, 3, 5, 7))
            # s1 GAP: quarter reduces chasing the split x1 DMAs — ACT
            # halves first (free then), DVE halves after chain(0,1).
            reduce_act_half(1, 0, 1)
            reduce_act_half(1, 1, 1)
            reduce_dve_half(1, 0, 0)
            reduce_dve_half(1, 1, 0)

            def weave_routing(s):
                lg = routing_logits_pe(s)
                z = routing_z(s, lg)
                zb = routing_bcast(s, z)
                routing_sigmoid(s, zb)

            def weave_s1():
                # s1 routing + chains; the ACT table never swaps now, so
                # chain(1,0) can take the full odd-expert assist set.
                weave_routing(1)
                combine_dve(1, 0, assist=(1, 3, 5, 7))
                combine_dve(1, 1)

            def weave_s23(s):
                def w():
                    weave_routing(s + 1)
                    combine_dve(s + 1, 0, assist=(1, 3, 5, 7))
                    combine_dve(s + 1, 1)
                return w

            def red4(s):
                # s's GAP as 4 quarter reduces on ACT, woven between
                # epilogues of an earlier phase-B.
                return {
                    2: lambda: reduce_act_half(s, 0, 0),
                    3: lambda: reduce_act_half(s, 0, 1),
                    4: lambda: reduce_act_half(s, 1, 0),
                    5: lambda: reduce_act_half(s, 1, 1),
                }

            conv_sample_phased(0, weave_bp={1: weave_s1, **red4(2)})
            conv_sample_phased(1, weave_a={5: weave_s23(1)}, weave_bp=red4(3))
            conv_sample_phased(2, weave_a={3: weave_s23(2)})
            conv_sample_phased(3)

    nc.compile()
    return nc


def _get_program():
    if "nc" not in _PROGRAM_CACHE:
        _PROGRAM_CACHE["nc"] = _build_program()
    return _PROGRAM_CACHE["nc"]


def kernel(x, routing_w, routing_b, kernel_weights, bn_gamma, bn_beta, bn_mean, bn_var,
           _trace=False, _trace_kwargs=None):
    x = np.asarray(x, dtype=np.float32)
    routing_w = np.asarray(routing_w, dtype=np.float32)
    routing_b = np.asarray(routing_b, dtype=np.float32)
    kernel_weights = np.asarray(kernel_weights, dtype=np.float32)
    bn_gamma = np.asarray(bn_gamma, dtype=np.float32)
    bn_beta = np.asarray(bn_beta, dtype=np.float32)
    bn_mean = np.asarray(bn_mean, dtype=np.float32)
    bn_var = np.asarray(bn_var, dtype=np.float32)

    kwb = kernel_weights.astype(NPBF16)
    kw7 = kwb.reshape(E, OT, 128, IT, 128, KS, KS)
    wt_host = []
    for ot in range(OT):
        # [E, IT, 128cin, KS, KS, 128cout] -> [E, IT, 128, SLAB]
        a = np.ascontiguousarray(kw7[:, ot].transpose(0, 2, 3, 4, 5, 1)).reshape(
            E, IT, 128, SLAB
        )
        # pair-interleave: [IT, NPAIR, 128, 2*SLAB]
        b_ = a.reshape(NPAIR, 2, IT, 128, SLAB).transpose(2, 0, 3, 1, 4)
        wt_host.append(np.ascontiguousarray(b_).reshape(IT, NPAIR, 128, 2 * SLAB))

    rwt_host = np.ascontiguousarray(
        routing_w.T.reshape(IT, 128, E).transpose(1, 0, 2)
    ).astype(NPBF16)
    cst_host = np.zeros((1, 128 + E), dtype=np.float32)
    cst_host[0, :128] = 1.0
    cst_host[0, 128:] = routing_b
    inv = bn_gamma / np.sqrt(bn_var + BN_EPS)
    bnb_full = bn_beta - bn_mean * inv
    bns_host = [
        np.ascontiguousarray(inv[ot * 128 : (ot + 1) * 128]).reshape(128, 1)
        for ot in range(OT)
    ]
    bnb_host = [
        np.ascontiguousarray(bnb_full[ot * 128 : (ot + 1) * 128]).reshape(128, 1)
        for ot in range(OT)
    ]

    jc_host = np.zeros((128, 448), dtype=NPBF16)
    jc_host[:, :128] = np.eye(128, dtype=np.float32).astype(NPBF16)
    jc_host[0, 128:256] = NPBF16(1.0)
    jc_host[:, 256:] = NPBF16(0.25)

    x_pad = np.zeros((B, CIN, H, WP), dtype=NPBF16)
    x_pad[:, :, :, 1 : 1 + W] = x.astype(NPBF16)
    # [B, CIN, H, WP] -> per group [SPC, 128, IT*H*WP] (partition-major)
    x_host = [
        np.ascontiguousarray(
            x_pad[g * SPC : (g + 1) * SPC]
            .reshape(SPC, IT, 128, H, WP)
            .transpose(0, 2, 1, 3, 4)
        ).reshape(SPC, 128, IT * H * WP)
        for g in range(B // SPC)
    ]

    in_maps = []
    for c in range(NCORES):
        ot = c % 2
        g = c // 2
        in_maps.append(
            {
                "jc": jc_host,
                "x": x_host[g],
                "wt": wt_host[ot],
                "rwt": rwt_host,
                "cst": cst_host,
                "bns": bns_host[ot],
                "bnb": bnb_host[ot],
            }
        )

    nc = _get_program()
    res = run_bass_kernel_spmd(
        nc, in_maps, core_ids=list(range(NCORES)),
        trace=_trace, **(_trace_kwargs or {}),
    )
    _PROGRAM_CACHE["last_result"] = res

    out = np.empty((B, COUT, H, W), dtype=np.float32)
    for c in range(NCORES):
        ot = c % 2
        g = c // 2
        yg = res.results[c]["y"]
        out[g * SPC : (g + 1) * SPC, ot * 128 : (ot + 1) * 128] = np.asarray(
            yg
        ).astype(np.float32)
    return out


# revision 58
# speedup vs baseline: 1.0226x; 1.0139x over previous
"""CondConv (MoE routed conv) Trainium2 Bass kernel — v4.

Sharding: core c -> samples [4*(c//2), 4*(c//2)+4), cout half ot = c%2.

Changes vs v3 (135992 ns):
  - All input DMA on the sync ring in strict priority order: gate const,
    x0 quarters, routing smalls, it0 slab pairs, x1, it1 slab pairs,
    x2, x3.  Slabs are loaded as expert PAIRS (host pre-interleaved) so
    the combine chain can chase arrivals; y output DMAs follow on the
    same ring.
  - No gpsimd DMA ring (its swdge-init memsets started the measured
    exec window ~1.1us early); nothing executes before the first
    DIRECT2D.
  - GAP for s0 runs as 4 quarter reduces on DVE with bf16 output
    (2x perf mode), chasing the x0 quarter DMAs; routing runs ~7us
    earlier than v3.
  - s0 combine chains chase the slab-pair arrivals; remaining samples
    use pure-DVE chains (ACT keeps epilogue slack).
  - Routing matmuls for s1..s3 are woven into the conv phase-A stream.
"""

import sys

sys.path.insert(0, "/opt/trn_rl_repo")

import ml_dtypes
import numpy as np

import concourse.bass as bass  # noqa: F401
import concourse.mybir as mybir
import concourse.tile as tile
from concourse import bacc
from concourse.bass_utils import run_bass_kernel_spmd

F32 = mybir.dt.float32
BF16 = mybir.dt.bfloat16
AF = mybir.ActivationFunctionType
ALU = mybir.AluOpType
NPBF16 = ml_dtypes.bfloat16

B, CIN, H, W = 16, 256, 56, 56
E, COUT, KS = 8, 256, 3
NCORES = 8
SPC = 4
IT = CIN // 128
OT = COUT // 128
KHKW = KS * KS
HB = 8
WP = W + 2
PIX = H * W
BN_EPS = 1e-5
SLAB = KHKW * 128  # 1152
NPAIR = E // 2
NPA = 7
JUNK_A = 22
JUNK_M = 2
JUNK_B = 2
HHALF = H // 2
HWP = H * WP

_PROGRAM_CACHE = {}


def _build_program():
    nc = bacc.Bacc("TRN2", target_bir_lowering=False, debug=False)

    jc_d = nc.dram_tensor("jc", [128, 448], BF16, kind="ExternalInput")
    # x is partition-major: each partition's IT*H*WP elements contiguous,
    # so DMA descriptors are one long run per partition (quarters = flat
    # column ranges), not 116-byte rows.
    x_d = nc.dram_tensor("x", [SPC, 128, IT * H * WP], BF16, kind="ExternalInput")
    wt_d = nc.dram_tensor("wt", [IT, NPAIR, 128, 2 * SLAB], BF16, kind="ExternalInput")
    rwt_d = nc.dram_tensor("rwt", [128, IT, E], BF16, kind="ExternalInput")
    # cst cols 0:128 = ones (bcast lhsT), cols 128:128+E = routing bias
    cst_d = nc.dram_tensor("cst", [1, 128 + E], F32, kind="ExternalInput")
    bns_d = nc.dram_tensor("bns", [128, 1], F32, kind="ExternalInput")
    bnb_d = nc.dram_tensor("bnb", [128, 1], F32, kind="ExternalInput")
    y_d = nc.dram_tensor("y", [SPC, 128, H, W], BF16, kind="ExternalOutput")

    with tile.TileContext(nc) as tc:
        with (
            tc.tile_pool(name="xp", bufs=1) as xp,
            tc.tile_pool(name="cwp", bufs=1) as cwp,
            tc.tile_pool(name="wtp", bufs=1) as wtp,
            tc.tile_pool(name="outp", bufs=6) as outp,
            tc.tile_pool(name="smal", bufs=1) as smal,
            tc.tile_pool(name="psc", bufs=NPA, space="PSUM") as psc,
            tc.tile_pool(name="pss", bufs=1, space="PSUM") as pss,
        ):
            # ---- priority-ordered input DMA, all on the sync ring ----
            jc = smal.tile([128, 448], BF16, tag="jc")
            nc.sync.dma_start(jc[:], jc_d[:])

            x_sb = {}
            x_flat = {}
            for s in range(SPC):
                x_sb[s] = xp.tile([128, IT, H, WP], BF16, tag=f"x{s}", name=f"x{s}")
                x_flat[s] = x_sb[s][:].rearrange("p a b c -> p (a b c)")

            QF = HHALF * WP  # flat elems per (it, H-half) quarter
            # x0 row-thirds per it (6 pieces) for a finer GAP chase
            THIRDS = [(0, 18), (18, 37), (37, H)]

            def load_x_rows(s, it, r0, r1):
                c0 = it * HWP + r0 * WP
                c1 = it * HWP + r1 * WP
                nc.sync.dma_start(x_flat[s][:, c0:c1], x_d[s, :, c0:c1])

            def load_x_quarter(s, it, q):
                load_x_rows(s, it, q * HHALF, (q + 1) * HHALF)

            for it in range(IT):
                for q in range(2):
                    load_x_quarter(0, it, q)

            rwt_sb = smal.tile([128, IT, E], BF16, tag="rwt")
            nc.sync.dma_start(rwt_sb[:], rwt_d[:])
            cst_sb = smal.tile([1, 128 + E], F32, tag="cst")
            nc.sync.dma_start(cst_sb[:], cst_d[:])
            ones_sb = cst_sb[:, 0:128]
            rb_sb = cst_sb[:, 128 : 128 + E]

            pair_tiles = {}
            DG0 = 512

            def load_slab_pairs(it, split=False):
                for p in range(NPAIR):
                    t = wtp.tile(
                        [128, 2 * SLAB], BF16, tag=f"wt{it}{p}", name=f"wt{it}{p}"
                    )
                    if split:
                        # head cols [0:DG0) of both experts first (the DVE
                        # chain region -> no arrival chase), tails after
                        # (the PE diag region chases them).
                        tv = t[:].rearrange("q (e c) -> q e c", e=2)
                        sv = wt_d[it, p].rearrange("q (e c) -> q e c", e=2)
                        nc.sync.dma_start(tv[:, :, 0:DG0], sv[:, :, 0:DG0])
                        nc.sync.dma_start(tv[:, :, DG0:], sv[:, :, DG0:])
                    else:
                        nc.sync.dma_start(t[:], wt_d[it, p])
                    pair_tiles[it, p] = t

            def slab(it, e):
                t = pair_tiles[it, e // 2]
                off = (e % 2) * SLAB
                return t[:, off : off + SLAB]

            load_slab_pairs(0)
            load_slab_pairs(1)
            bns_sb = smal.tile([128, 1], F32, tag="bns")
            nc.sync.dma_start(bns_sb[:], bns_d[:])
            bnb_sb = smal.tile([128, 1], F32, tag="bnb")
            nc.sync.dma_start(bnb_sb[:], bnb_d[:])
            # x1 split in quarters so its GAP reduces can chase
            for it in range(IT):
                for q in range(2):
                    load_x_quarter(1, it, q)
            nc.sync.dma_start(x_flat[2][:], x_d[2])
            nc.sync.dma_start(x_flat[3][:], x_d[3])

            # ---- ACT table preload (gated on jc, not t=0). Silu is the
            # ONLY table the kernel ever uses: routing computes
            # sigmoid(z) as silu(z)/z so the ACT engine never swaps
            # tables (a swap costs 1.28us and landed on the routing
            # critical path).
            sig_dummy = smal.tile([1, 8], BF16, tag="sigd")
            nc.scalar.activation(sig_dummy[:], jc[0:1, 0:8], AF.Silu)

            def warmup(n):
                for _ in range(n):
                    jps = psc.tile([128, HB, W], F32, tag="ps", name="jps")
                    flat = jps[:].rearrange("p a b -> p (a b)")
                    nc.tensor.matmul(
                        flat[:, 0:448], jc[:, 0:128], jc[:], start=True, stop=True
                    )

            # ---- routing ----
            pooled = {}  # (s, it, piece) -> [128,1] bf16 partial sums
            r_bcast = {}

            # Two-stage GAP: X-only reduce to [128, rows] keeps the DVE
            # 2x packed mode (scalar-per-partition dst forces 1x), then a
            # tiny second reduce to [128, 1].
            def reduce_dve(s, it, q, rows):
                r0, r1 = rows
                rs = smal.tile(
                    [128, H], BF16, tag=f"rs{s}{it}{q}", name=f"rs{s}{it}{q}"
                )
                p = smal.tile(
                    [128, 1], BF16, tag=f"po{s}{it}{q}", name=f"po{s}{it}{q}"
                )
                with nc.allow_low_precision(reason="GAP partials feed sigmoid"):
                    nc.vector.reduce_sum(
                        rs[:, : r1 - r0],
                        x_sb[s][:, it, r0:r1, :],
                        axis=mybir.AxisListType.X,
                    )
                    nc.vector.reduce_sum(
                        p[:], rs[:, : r1 - r0], axis=mybir.AxisListType.X
                    )
                pooled[s, it, q] = p

            def reduce_dve_half(s, it, q):
                reduce_dve(s, it, q, (q * HHALF, (q + 1) * HHALF))

            def reduce_dve_full(s, it):
                reduce_dve(s, it, 0, (0, H))

            def reduce_act(s, it, q, rows):
                r0, r1 = rows
                p = smal.tile(
                    [128, 1], BF16, tag=f"po{s}{it}{q}", name=f"po{s}{it}{q}"
                )
                sl = x_sb[s][:, it, r0:r1, :]
                with nc.allow_low_precision(reason="GAP partials feed sigmoid"):
                    nc.scalar.activation(sl, sl, AF.Copy, accum_out=p[:])
                pooled[s, it, q] = p

            def reduce_act_half(s, it, q):
                reduce_act(s, it, q, (q * HHALF, (q + 1) * HHALF))

            def reduce_act_full(s, it):
                reduce_act(s, it, 0, (0, H))

            def routing_logits_pe(s):
                lg_ps = pss.tile([1, E], F32, tag="rps", name=f"lgps{s}")
                parts = [k for k in pooled if k[0] == s]
                for i, (s_, it, q) in enumerate(parts):
                    nc.tensor.matmul(
                        lg_ps[:], pooled[s_, it, q][:], rwt_sb[:, it],
                        start=(i == 0), stop=(i == len(parts) - 1),
                    )
                return lg_ps

            def routing_z(s, lg_ps):
                zr = smal.tile([1, E], F32, tag=f"z{s}", name=f"z{s}")
                nc.vector.scalar_tensor_tensor(
                    zr[:], lg_ps[:], 1.0 / PIX, rb_sb, ALU.mult, ALU.add
                )
                return zr

            def routing_bcast(s, zr):
                zb_ps = pss.tile([128, E], F32, tag="rps", name=f"zbps{s}")
                nc.tensor.matmul(zb_ps[:], ones_sb, zr[:], start=True, stop=True)
                return zb_ps

            def routing_sigmoid(s, zb_ps):
                # sigmoid(z) = silu(z) * (1/z)  (z==0 has measure zero;
                # the f32 ratio is numerically stable for small |z|).
                st = smal.tile([128, E], F32, tag=f"st{s}", name=f"st{s}")
                nc.scalar.activation(st[:], zb_ps[:], AF.Silu)
                zi = smal.tile([128, E], F32, tag=f"zi{s}", name=f"zi{s}")
                nc.vector.reciprocal(zi[:], zb_ps[:])
                rbc = smal.tile([128, E], F32, tag=f"rbc{s}", name=f"rbc{s}")
                nc.vector.tensor_tensor(rbc[:], st[:], zi[:], ALU.mult)
                r_bcast[s] = rbc

            cw_r = {
                (s, it): cwp.tile(
                    [128, SLAB], BF16, tag=f"cwr_{s}_{it}", name=f"cwr_{s}_{it}"
                )
                for s in range(SPC)
                for it in range(IT)
            }
            acc_a = cwp.tile([128, SLAB], BF16, tag="acc_a")
            acc_b = cwp.tile([128, SLAB], BF16, tag="acc_b")
            tmp_t = cwp.tile([128, SLAB], BF16, tag="tmp")
            u_a = cwp.tile([128, SLAB], BF16, tag="u_a")
            u_b = cwp.tile([128, SLAB], BF16, tag="u_b")

            def combine_dve(s, it, c0=0, c1=SLAB, assist=None):
                # TS + TT ping-pong chain in slab-pair arrival order over
                # cols [c0:c1).  Odd-e scale copies can run on the Scalar
                # ("act") or GpSimd ("gp") engine in parallel with the
                # DVE adds.
                dst = cw_r[s, it]
                accs = [acc_a, acc_b]
                us = [u_a, u_b]
                cur = None
                for e in range(E):
                    wt_ap = slab(it, e)[:, c0:c1]
                    sc = r_bcast[s][:, e : e + 1]
                    if e == 0:
                        cur = accs[0]
                        nc.vector.tensor_scalar_mul(cur[:, c0:c1], wt_ap, sc)
                        continue
                    if assist and e in assist:
                        u = us[(e // 2) % 2]
                        nc.scalar.activation(u[:, c0:c1], wt_ap, AF.Copy, scale=sc)
                    else:
                        u = tmp_t
                        nc.vector.tensor_scalar_mul(u[:, c0:c1], wt_ap, sc)
                    nxt = dst if e == E - 1 else accs[e % 2]
                    nc.vector.tensor_tensor(
                        nxt[:, c0:c1], cur[:, c0:c1], u[:, c0:c1], ALU.add
                    )
                    cur = nxt

            # PE combines cw(0,0) cols [DG0:SLAB) via diag-scaled matmuls
            # (diag tiles built on GpSimd from the identity in jc) while
            # the DVE chain covers [0:DG0).
            diag_tiles = [
                smal.tile([128, 128], BF16, tag=f"dg{e}", name=f"dg{e}")
                for e in range(E)
            ]

            def diag_builds():
                # On DVE: cheap [128,128] scales that fit in the combine
                # chain's slab-arrival gaps (ACT versions cost ~400ns each
                # and blocked the chain's scale assists).
                for e in range(E):
                    nc.vector.tensor_scalar_mul(
                        diag_tiles[e][:], jc[:, 0:128], r_bcast[0][:, e : e + 1]
                    )

            def combine_pe_diag():
                chunks = [(DG0, DG0 + 448), (DG0 + 448, SLAB)]
                pts = []
                for i, (c0, c1) in enumerate(chunks):
                    t = psc.tile([128, HB, W], F32, tag="ps", name=f"dgps{i}")
                    pts.append(t[:].rearrange("p a b -> p (a b)"))
                for e in range(E):
                    for i, (c0, c1) in enumerate(chunks):
                        nc.tensor.matmul(
                            pts[i][:, : c1 - c0], diag_tiles[e][:],
                            slab(0, e)[:, c0:c1],
                            start=(e == 0), stop=(e == E - 1),
                        )
                for i, (c0, c1) in enumerate(chunks):
                    nc.scalar.activation(
                        cw_r[0, 0][:, c0:c1], pts[i][:, : c1 - c0], AF.Copy
                    )

            hblocks = [(h0, min(HB, H - h0)) for h0 in range(0, H, HB)]
            # khkw >= 4 first: those cw columns come from the PE diag
            # combine and are ready before the DVE-chain columns
            # (khkw 0..3), so phase-A(s0) can start sooner.
            taps = [(0, 0), (0, 1), (1, -1), (1, 0), (1, 1),
                    (-1, -1), (-1, 0), (-1, 1), (0, -1)]

            def block_total(h0, nh):
                return IT * sum(
                    1 for dh, dw in taps if min(h0 + nh, H - dh) > max(h0, -dh)
                )

            def conv_block_taps(s, h0, nh, ps_t, its, n_mm, total):
                for it in its:
                    for dh, dw in taps:
                        khkw = (dh + 1) * 3 + (dw + 1)
                        ho_s = max(h0, -dh)
                        ho_e = min(h0 + nh, H - dh)
                        if ho_e <= ho_s:
                            continue
                        nhh = ho_e - ho_s
                        hi_s = ho_s + dh
                        off = khkw * 128
                        lhsT = cw_r[s, it][:, off : off + 128]
                        rhs = x_sb[s][:, it, hi_s : hi_s + nhh, 1 + dw : 1 + dw + W]
                        out = ps_t[:, ho_s - h0 : ho_s - h0 + nhh, 0:W]
                        nc.tensor.matmul(
                            out, lhsT, rhs,
                            start=(n_mm == 0), stop=(n_mm == total - 1),
                        )
                        n_mm += 1
                return n_mm

            def conv_epilogue(s, h0, nh, ps_t):
                o_t = outp.tile([128, HB, W], BF16, tag="out", name="o_t")
                nc.scalar.activation(
                    o_t[:, :nh, :], ps_t[:, :nh, :], AF.Silu,
                    bias=bnb_sb[:], scale=bns_sb[:],
                )
                nc.sync.dma_start(y_d[s, :, h0 : h0 + nh, :], o_t[:, :nh, :])

            def conv_sample_phased(s, weave_a=None, weave_bp=None):
                # weave_bp runs after phase-B block bi's taps but BEFORE
                # its epilogue, so a routing sigmoid emitted there lands
                # ahead of the Silu epilogues on the ACT queue.
                weave_a = weave_a or {}
                weave_bp = weave_bp or {}
                pa_s = []
                for bi, (h0, nh) in enumerate(hblocks):
                    ps_t = psc.tile([128, HB, W], F32, tag="ps", name="ps")
                    total = block_total(h0, nh)
                    n_mm = conv_block_taps(s, h0, nh, ps_t, [0], 0, total)
                    pa_s.append((h0, nh, ps_t, n_mm, total))
                    if bi in weave_a:
                        weave_a[bi]()
                for bi, (h0, nh, ps_t, n_mm, total) in enumerate(pa_s):
                    n_mm = conv_block_taps(s, h0, nh, ps_t, [1], n_mm, total)
                    assert n_mm == total
                    if bi in weave_bp:
                        weave_bp[bi]()
                    conv_epilogue(s, h0, nh, ps_t)

            # ================= emission schedule =================
            # s0 GAP: 4 quarter reduces chasing the x0 quarter DMAs,
            # alternating DVE / ACT so neither serializes.
            reduce_dve_half(0, 0, 0)
            reduce_act_half(0, 0, 1)
            reduce_dve_half(0, 1, 0)
            reduce_act_half(0, 1, 1)

            warmup(JUNK_A)
            lg0 = routing_logits_pe(0)
            z0 = routing_z(0, lg0)
            warmup(JUNK_M)
            zb0 = routing_bcast(0, z0)
            routing_sigmoid(0, zb0)
            warmup(JUNK_B)
            diag_builds()
            combine_pe_diag()

            combine_dve(0, 0, c1=DG0, assist=(1, 3, 5, 7))
            combine_dve(0, 1, assist=(1, 3, 5, 7))
            # s1 GAP: quarter reduces chasing the split x1 DMAs — ACT
            # halves first (free then), DVE halves after chain(0,1).
            reduce_act_half(1, 0, 1)
            reduce_act_half(1, 1, 1)
            reduce_dve_half(1, 0, 0)
            reduce_dve_half(1, 1, 0)

            def weave_routing(s):
                lg = routing_logits_pe(s)
                z = routing_z(s, lg)
                zb = routing_bcast(s, z)
                routing_sigmoid(s, zb)

            def weave_s1():
                # s1 routing + chains; the ACT table never swaps now, so
                # chain(1,0) can take the full odd-expert assist set.
                weave_routing(1)
                combine_dve(1, 0, assist=(1, 3, 5, 7))
                combine_dve(1, 1)

            def weave_s23(s):
                def w():
                    weave_routing(s + 1)
                    combine_dve(s + 1, 0, assist=(1, 3, 5, 7))
                    combine_dve(s + 1, 1)
                return w

            def red4(s):
                # s's GAP as 4 quarter reduces on ACT, woven between
                # epilogues of an earlier phase-B.
                return {
                    2: lambda: reduce_act_half(s, 0, 0),
                    3: lambda: reduce_act_half(s, 0, 1),
                    4: lambda: reduce_act_half(s, 1, 0),
                    5: lambda: reduce_act_half(s, 1, 1),
                }

            conv_sample_phased(0, weave_bp={1: weave_s1, **red4(2)})
            conv_sample_phased(1, weave_a={5: weave_s23(1)}, weave_bp=red4(3))
            conv_sample_phased(2, weave_a={3: weave_s23(2)})
            conv_sample_phased(3)

    nc.compile()
    return nc


def _get_program():
    if "nc" not in _PROGRAM_CACHE:
        _PROGRAM_CACHE["nc"] = _build_program()
    return _PROGRAM_CACHE["nc"]


def kernel(x, routing_w, routing_b, kernel_weights, bn_gamma, bn_beta, bn_mean, bn_var,
           _trace=False, _trace_kwargs=None):
    x = np.asarray(x, dtype=np.float32)
    routing_w = np.asarray(routing_w, dtype=np.float32)
    routing_b = np.asarray(routing_b, dtype=np.float32)
    kernel_weights = np.asarray(kernel_weights, dtype=np.float32)
    bn_gamma = np.asarray(bn_gamma, dtype=np.float32)
    bn_beta = np.asarray(bn_beta, dtype=np.float32)
    bn_mean = np.asarray(bn_mean, dtype=np.float32)
    bn_var = np.asarray(bn_var, dtype=np.float32)

    kwb = kernel_weights.astype(NPBF16)
    kw7 = kwb.reshape(E, OT, 128, IT, 128, KS, KS)
    wt_host = []
    for ot in range(OT):
        # [E, IT, 128cin, KS, KS, 128cout] -> [E, IT, 128, SLAB]
        a = np.ascontiguousarray(kw7[:, ot].transpose(0, 2, 3, 4, 5, 1)).reshape(
            E, IT, 128, SLAB
        )
        # pair-interleave: [IT, NPAIR, 128, 2*SLAB]
        b_ = a.reshape(NPAIR, 2, IT, 128, SLAB).transpose(2, 0, 3, 1, 4)
        wt_host.append(np.ascontiguousarray(b_).reshape(IT, NPAIR, 128, 2 * SLAB))

    rwt_host = np.ascontiguousarray(
        routing_w.T.reshape(IT, 128, E).transpose(1, 0, 2)
    ).astype(NPBF16)
    cst_host = np.zeros((1, 128 + E), dtype=np.float32)
    cst_host[0, :128] = 1.0
    cst_host[0, 128:] = routing_b
    inv = bn_gamma / np.sqrt(bn_var + BN_EPS)
    bnb_full = bn_beta - bn_mean * inv
    bns_host = [
        np.ascontiguousarray(inv[ot * 128 : (ot + 1) * 128]).reshape(128, 1)
        for ot in range(OT)
    ]
    bnb_host = [
        np.ascontiguousarray(bnb_full[ot * 128 : (ot + 1) * 128]).reshape(128, 1)
        for ot in range(OT)
    ]

    jc_host = np.zeros((128, 448), dtype=NPBF16)
    jc_host[:, :128] = np.eye(128, dtype=np.float32).astype(NPBF16)
    jc_host[0, 128:256] = NPBF16(1.0)
    jc_host[:, 256:] = NPBF16(0.25)

    x_pad = np.zeros((B, CIN, H, WP), dtype=NPBF16)
    x_pad[:, :, :, 1 : 1 + W] = x.astype(NPBF16)
    # [B, CIN, H, WP] -> per group [SPC, 128, IT*H*WP] (partition-major)
    x_host = [
        np.ascontiguousarray(
            x_pad[g * SPC : (g + 1) * SPC]
            .reshape(SPC, IT, 128, H, WP)
            .transpose(0, 2, 1, 3, 4)
        ).reshape(SPC, 128, IT * H * WP)
        for g in range(B // SPC)
    ]

    in_maps = []
    for c in range(NCORES):
        ot = c % 2
        g = c // 2
        in_maps.append(
            {
                "jc": jc_host,
                "x": x_host[g],
                "wt": wt_host[ot],
                "rwt": rwt_host,
                "cst": cst_host,
                "bns": bns_host[ot],
                "bnb": bnb_host[ot],
            }
        )

    nc = _get_program()
    res = run_bass_kernel_spmd(
        nc, in_maps, core_ids=list(range(NCORES)),
        trace=_trace, **(_trace_kwargs or {}),
    )
    _PROGRAM_CACHE["last_result"] = res

    out = np.empty((B, COUT, H, W), dtype=np.float32)
    for c in range(NCORES):
        ot = c % 2
        g = c // 2
        yg = res.results[c]["y"]
        out[g * SPC : (g + 1) * SPC, ot * 128 : (ot + 1) * 128] = np.asarray(
            yg
        ).astype(np.float32)
    return out
